# revision 52
# baseline (speedup 1.0000x reference)
"""Correlation volume (9x9 displacements) kernel for 8 Trainium2 NeuronCores.

input1, input2: [8, 256, 128, 128] f32  ->  out [8, 9, 9, 128, 128] f32
Data-parallel over batch N: core n computes batch element n.

Self-contained: builds and compiles the Bass kernel on first call.
"""
import sys
if '/opt/trn_rl_repo' not in sys.path:
    sys.path.insert(0, '/opt/trn_rl_repo')

import numpy as np
import concourse.bass as bass
import concourse.mybir as mybir
import concourse.masks as masks
from concourse.ap import AP
from concourse.tile import TileContext

# ---- workaround: this walrus build allows only 1 semaphore wait per
# instruction; split Tile's multi-wait instructions into nop-carried
# single waits, and the tail drain likewise ----
import concourse.tile as _tile
import concourse.bass2jax as _b2j
import concourse.bass_utils as _bu
from concourse.vector_clock import ScopedClock as _ScopedClock

def _patched_drain_and_barrier(self, tick_clock, wait_clock):
    nc = self.nc
    probe = nc.sync.nop(nofuse=True)
    wait_clock.add_sem_waits(probe.ins, _ScopedClock({None: tick_clock.global_clock}))
    waits = list(probe.ins.sync_info.on_wait or [])
    probe.ins.sync_info.on_wait = waits[:1]
    rest = waits[1:]
    while rest:
        nop = nc.sync.nop(nofuse=True)
        if nop.ins.sync_info is None:
            nop.ins.sync_info = mybir.SyncInfo(on_wait=[], on_update=[])
        nop.ins.sync_info.on_wait = rest[:1]
        rest = rest[1:]
    nc.sync.drain()
    nc.all_engine_barrier()
    assert self.sems is not None
    popped = nc._tile_sem_poison_stack.pop()
    assert popped is self._sem_poison
    nc.clear_and_free_semaphores(list(self.sems.allocated().values()))
    nc.all_engine_barrier()

_tile.TileContext._drain_and_barrier = _patched_drain_and_barrier

_nop_counter = [0]

def _split_multi_waits(nc):
    for f in nc.m.functions:
        for blk in f.blocks:
            insts, outl, changed = list(blk.instructions), [], False
            for inst in insts:
                si = inst.sync_info
                if si is not None and si.on_wait and len(si.on_wait) > 1:
                    waits = list(si.on_wait)
                    for w in waits[:-1]:
                        _nop_counter[0] += 1
                        outl.append(mybir.InstNoOp(
                            name=f"waitsplit-{_nop_counter[0]}",
                            engine=inst.engine,
                            sync_info=mybir.SyncInfo(on_wait=[w], on_update=[])))
                    si.on_wait = waits[-1:]
                    inst.sync_info = si
                    changed = True
                outl.append(inst)
            if changed:
                blk.instructions = outl

_orig_run_via_pjrt = _b2j.run_bass_via_pjrt

def _patched_run_via_pjrt(nc, in_maps, **kw):
    _split_multi_waits(nc)
    return _orig_run_via_pjrt(nc, in_maps, **kw)

_b2j.run_bass_via_pjrt = _patched_run_via_pjrt
_bu.bass2jax = _b2j
_bu.upload_artifacts = lambda tmpdir: tmpdir

F32 = mybir.dt.float32
BF16 = mybir.dt.bfloat16

C, H, W = 256, 128, 128
CH = 2              # contraction chunks of 128
PATCH = 9
NQ = 81
YC = 16             # y rows per chunk
SLOTS = YC + 8      # 24 in2 rows resident per chunk (y0-4 .. y0+19)
XP = W + 8          # 136 padded x positions per row
NBLK, B, WXL = 4, 32, 40   # x-blocks: base 32b, 32 pixels, 40-col window
NBAND = WXL * PATCH        # 360 band cols per row
BW = 384                   # padded band row width
ROW_F = SLOTS * XP         # 3264 in2b elems per c-chunk
G = 288                    # guard cols at the front of the skew buffer
R2 = G + YC * BW           # 6432: skew buffer row width (bf16)


def ap3(t, off, dims):
    return AP(tensor=t[:].tensor, offset=t[:].offset + off, ap=dims)


def build(nc: bass.Bass, ycap=H):
    in1 = nc.declare_dram_parameter("in1", [C, H, W], F32, isOutput=False)
    in2 = nc.declare_dram_parameter("in2", [C, H, W], F32, isOutput=False)
    out = nc.declare_dram_parameter("out", [NQ, H, W], BF16, isOutput=True)

    nchunk = (ycap + YC - 1) // YC

    with TileContext(nc) as tc:
        with tc.tile_pool(name="const", bufs=1) as cpool, \
             tc.tile_pool(name="in1f", bufs=3) as p_in1f, \
             tc.tile_pool(name="in2f", bufs=3) as p_in2f, \
             tc.tile_pool(name="in1b", bufs=2) as p_in1b, \
             tc.tile_pool(name="in2b", bufs=2) as p_in2b, \
             tc.tile_pool(name="bandbuf", bufs=3) as p_band, \
             tc.tile_pool(name="bandrd", bufs=2) as p_c, \
             tc.tile_pool(name="t32", bufs=3) as p_t32, \
             tc.tile_pool(name="band_ps", bufs=5, space="PSUM") as p_ps, \
             tc.tile_pool(name="t_ps", bufs=2, space="PSUM") as p_tps, \
             tc.tile_pool(name="band_dram", bufs=3, space="DRAM") as p_bd:

            def issue_loads(k):
                y0 = k * YC
                in1f = p_in1f.tile([128, CH * YC * W], F32)
                nc.sync.dma_start(
                    out=in1f[:],
                    in_=ap3(in1, y0 * W,
                            [[H * W, 128], [128 * H * W, CH], [1, YC * W]]))
                new0, s0 = (0, 4) if k == 0 else (y0 + 4, 8)
                nrow = min(H, new0 + SLOTS - s0) - new0
                in2f = None
                if nrow > 0:
                    in2f = p_in2f.tile([128, CH * 20 * W], F32)
                    nc.sync.dma_start(
                        out=ap3(in2f, 0,
                                [[CH * 20 * W, 128], [20 * W, CH], [1, nrow * W]]),
                        in_=ap3(in2, new0 * W,
                                [[H * W, 128], [128 * H * W, CH], [1, nrow * W]]))
                return in1f, in2f, new0, s0, nrow

            def build_inputs(k):
                in1f, in2f, new0, s0, nrow = loads.pop(k)
                in1b = p_in1b.tile([128, CH * YC * W], BF16)
                nc.scalar.copy(out=in1b[:], in_=in1f[:])
                # ---- in2 row-major padded buffer: rows y0-4 .. y0+19 ----
                in2b = p_in2b.tile([128, CH * ROW_F], BF16)
                for ch in range(CH):
                    nc.vector.memset(
                        ap3(in2b, ch * ROW_F, [[CH * ROW_F, 128], [1, 4 * SLOTS]]), 0)
                    nc.vector.memset(
                        ap3(in2b, ch * ROW_F + (4 + W) * SLOTS,
                            [[CH * ROW_F, 128], [1, 4 * SLOTS]]), 0)
                if k == 0:
                    for ch in range(CH):
                        nc.vector.memset(
                            ap3(in2b, ch * ROW_F, [[CH * ROW_F, 128], [SLOTS, XP], [1, 4]]), 0)
                else:
                    nc.vector.tensor_copy(
                        ap3(in2b, 0, [[CH * ROW_F, 128], [ROW_F, CH], [SLOTS, XP], [1, 8]]),
                        ap3(prev_in2b[0], 16,
                            [[CH * ROW_F, 128], [ROW_F, CH], [SLOTS, XP], [1, 8]]))
                if nrow > 0:
                    for ch in range(CH):
                        nc.vector.tensor_copy(
                            ap3(in2b, ch * ROW_F + 4 * SLOTS + s0,
                                [[CH * ROW_F, 128], [SLOTS, W], [1, nrow]]),
                            ap3(in2f, ch * 20 * W,
                                [[CH * 20 * W, 128], [1, W], [W, nrow]]))
                stail = s0 + nrow
                if stail < SLOTS:
                    for ch in range(CH):
                        nc.vector.memset(
                            ap3(in2b, ch * ROW_F + stail,
                                [[CH * ROW_F, 128], [SLOTS, XP], [1, SLOTS - stail]]), 0)
                prev_in2b[0] = in2b
                return in1b, in2b

            prepared = {}
            ident = cpool.tile([128, 128], BF16)
            masks.make_identity(nc, ident[:])
            loads = {0: issue_loads(0)}
            if nchunk > 1:
                loads[1] = issue_loads(1)
            prev_in2b = [None]
            deferred = None
            for k in range(nchunk):
                y0 = k * YC
                if k + 2 < nchunk:
                    loads[k + 2] = issue_loads(k + 2)
                if k in prepared:
                    in1b, in2b = prepared.pop(k)
                else:
                    in1b, in2b = build_inputs(k)

                # ---- per row: matmuls + band copy ----
                bandbuf = p_band.tile([128, YC * BW], BF16)
                bandrd = p_c.tile([128, YC * 128], BF16)
                t32 = p_t32.tile([128, YC * 128], BF16)
                thalf = 0
                band_dram = p_bd.tile([128, YC * BW], BF16)
                if k < 3:
                    # first use of each bandbuf: init the per-slot junk cols
                    # (360..383) that the full-row band write reads
                    nc.vector.memset(
                        ap3(bandbuf, NBAND,
                            [[YC * BW, 128], [BW, YC], [1, BW - NBAND]]), 0)
                if k < 3:
                    # first use of each readback buffer: init the junk cols
                    # (81..127 per slot) the transposes read but the 81-col
                    # readback never writes
                    nc.vector.memset(
                        ap3(bandrd, 81, [[YC * 128, 128], [128, YC], [1, 128 - 81]]),
                        0)

                for t in range(YC):
                    if y0 + t >= ycap:
                        break
                    # interleave previous chunk's transposes into this row loop
                    if deferred is not None and t % 4 == 2:
                        d_bandrd, d_t32, d_y0, d_ylim, d_th, d_k = deferred
                        tg = (t // 4) * 4
                        if tg < d_ylim:
                            tps = p_tps.tile([128, 512], BF16)
                            for dt in range(4):
                                if tg + dt >= d_ylim:
                                    break
                                nc.tensor.transpose(
                                    tps[:, dt * 128:(dt + 1) * 128],
                                    d_bandrd[:, (tg + dt) * 128:(tg + dt + 1) * 128],
                                    ident[:])
                            nc.vector.tensor_copy(
                                ap3(d_t32, d_th + tg * 128, [[YC * 128, 81], [1, 512]]),
                                tps[0:81, :])
                    ps = p_ps.tile([128, 512], F32)
                    for b in range(NBLK):
                        xb = 32 * b
                        for ch in range(CH):
                            lhsT = ap3(in1b, ch * YC * W + t * W + xb,
                                       [[CH * YC * W, 128], [1, B]])
                            # rhs col (xl, dy) = in2b row t+dy, xp xb+xl
                            rhs = ap3(in2b, ch * ROW_F + xb * SLOTS + t,
                                      [[CH * ROW_F, 128], [SLOTS, WXL], [1, PATCH]])
                            pso = AP(tensor=ps[:].tensor,
                                     offset=ps[:].offset + xb * 512,
                                     ap=[[512, B], [1, NBAND]])
                            nc.tensor.matmul(pso, lhsT, rhs,
                                             start=(ch == 0), stop=(ch == CH - 1),
                                             tile_position=(0, xb))
                    if t % 16 < 7:
                        nc.vector.tensor_copy(
                            ap3(bandbuf, t * BW, [[YC * BW, 128], [1, NBAND]]),
                            ps[:, 0:NBAND])
                    else:
                        nc.scalar.copy(
                            out=ap3(bandbuf, t * BW, [[YC * BW, 128], [1, NBAND]]),
                            in_=ps[:, 0:NBAND])
                    if t == 7:
                        # first-half band write overlaps rows 8-15
                        write_band(nc, band_dram, bandbuf, 0, 8)

                # ---- second-half band write + skewed 81-col read back ----
                write_band(nc, band_dram, bandbuf, 8, 8)
                rb_eng = nc.gpsimd if k < 5 else nc.sync
                for b in range(NBLK):
                    rb_eng.dma_start(
                        out=ap3(bandrd, b * B * YC * 128,
                                [[YC * 128, B], [128, YC], [1, 81]]),
                        in_=ap3(band_dram, b * B * YC * BW,
                                [[YC * BW + 9, B], [BW, YC], [1, 81]]))

                # ---- flush previous chunk's outputs ----
                if deferred is not None:
                    flush_out(nc, out, deferred)
                deferred = (bandrd, t32, y0, min(YC, ycap - y0), thalf, k)
                if k + 1 < nchunk:
                    prepared[k + 1] = build_inputs(k + 1)

            # final chunk: transposes + outputs
            if deferred is not None:
                d_bandrd, d_t32, d_y0, d_ylim, d_th, d_k = deferred
                for tg in range(0, d_ylim, 4):
                    tps = p_tps.tile([128, 512], BF16)
                    for dt in range(4):
                        if tg + dt >= d_ylim:
                            break
                        nc.tensor.transpose(
                            tps[:, dt * 128:(dt + 1) * 128],
                            d_bandrd[:, (tg + dt) * 128:(tg + dt + 1) * 128],
                            ident[:])
                    nc.vector.tensor_copy(
                        ap3(d_t32, d_th + tg * 128, [[YC * 128, 81], [1, 512]]),
                        tps[0:81, :])
                flush_out(nc, out, deferred)
    return nc


def write_band(nc, band_dram, bandbuf, yh0, nyr):
    nc.scalar.dma_start(
        out=ap3(band_dram, yh0 * BW, [[YC * BW, 128], [1, nyr * BW]]),
        in_=ap3(bandbuf, yh0 * BW, [[YC * BW, 128], [1, nyr * BW]]))


def flush_out(nc, out, deferred):
    d_bandrd, d_t32, d_y0, d_ylim, d_th, d_k = deferred
    fl_eng = nc.gpsimd if d_k < 5 else nc.sync
    for u in range(PATCH):
        fl_eng.dma_start(
            out=ap3(out, u * H * W + d_y0 * W,
                    [[PATCH * H * W, PATCH], [W, d_ylim], [1, W]]),
            in_=ap3(d_t32, (PATCH * u) * YC * 128,
                    [[YC * 128, PATCH], [128, d_ylim], [1, 128]]))


_cached_nc = [None]


def _get_nc():
    if _cached_nc[0] is None:
        nc = bass.Bass()
        build(nc)
        _cached_nc[0] = nc
    return _cached_nc[0]


def kernel(input1: np.ndarray, input2: np.ndarray, _trace=False) -> np.ndarray:
    from concourse.bass_utils import run_bass_kernel_spmd
    nc = _get_nc()
    n = input1.shape[0]
    in_maps = [{"in1": np.ascontiguousarray(input1[i], dtype=np.float32),
                "in2": np.ascontiguousarray(input2[i], dtype=np.float32)}
               for i in range(n)]
    r = run_bass_kernel_spmd(nc, in_maps, core_ids=list(range(n)), trace=_trace)
    outs = np.stack([r.results[i]["out"].reshape(9, 9, 128, 128) for i in range(n)])
    if _trace:
        return outs.astype(np.float32), r
    return outs.astype(np.float32)



# revision 53
# speedup vs baseline: 1.0103x; 1.0103x over previous
"""Correlation volume (9x9 displacements) kernel for 8 Trainium2 NeuronCores.

input1, input2: [8, 256, 128, 128] f32  ->  out [8, 9, 9, 128, 128] f32
Data-parallel over batch N: core n computes batch element n.

Self-contained: builds and compiles the Bass kernel on first call.
"""
import sys
if '/opt/trn_rl_repo' not in sys.path:
    sys.path.insert(0, '/opt/trn_rl_repo')

import numpy as np
import concourse.bass as bass
import concourse.mybir as mybir
import concourse.masks as masks
from concourse.ap import AP
from concourse.tile import TileContext

# ---- workaround: this walrus build allows only 1 semaphore wait per
# instruction; split Tile's multi-wait instructions into nop-carried
# single waits, and the tail drain likewise ----
import concourse.tile as _tile
import concourse.bass2jax as _b2j
import concourse.bass_utils as _bu
from concourse.vector_clock import ScopedClock as _ScopedClock

def _patched_drain_and_barrier(self, tick_clock, wait_clock):
    nc = self.nc
    probe = nc.sync.nop(nofuse=True)
    wait_clock.add_sem_waits(probe.ins, _ScopedClock({None: tick_clock.global_clock}))
    waits = list(probe.ins.sync_info.on_wait or [])
    probe.ins.sync_info.on_wait = waits[:1]
    rest = waits[1:]
    while rest:
        nop = nc.sync.nop(nofuse=True)
        if nop.ins.sync_info is None:
            nop.ins.sync_info = mybir.SyncInfo(on_wait=[], on_update=[])
        nop.ins.sync_info.on_wait = rest[:1]
        rest = rest[1:]
    nc.sync.drain()
    nc.all_engine_barrier()
    assert self.sems is not None
    popped = nc._tile_sem_poison_stack.pop()
    assert popped is self._sem_poison
    nc.clear_and_free_semaphores(list(self.sems.allocated().values()))
    nc.all_engine_barrier()

_tile.TileContext._drain_and_barrier = _patched_drain_and_barrier

_nop_counter = [0]

def _split_multi_waits(nc):
    for f in nc.m.functions:
        for blk in f.blocks:
            insts, outl, changed = list(blk.instructions), [], False
            for inst in insts:
                si = inst.sync_info
                if si is not None and si.on_wait and len(si.on_wait) > 1:
                    waits = list(si.on_wait)
                    for w in waits[:-1]:
                        _nop_counter[0] += 1
                        outl.append(mybir.InstNoOp(
                            name=f"waitsplit-{_nop_counter[0]}",
                            engine=inst.engine,
                            sync_info=mybir.SyncInfo(on_wait=[w], on_update=[])))
                    si.on_wait = waits[-1:]
                    inst.sync_info = si
                    changed = True
                outl.append(inst)
            if changed:
                blk.instructions = outl

_orig_run_via_pjrt = _b2j.run_bass_via_pjrt

def _patched_run_via_pjrt(nc, in_maps, **kw):
    _split_multi_waits(nc)
    return _orig_run_via_pjrt(nc, in_maps, **kw)

_b2j.run_bass_via_pjrt = _patched_run_via_pjrt
_bu.bass2jax = _b2j
_bu.upload_artifacts = lambda tmpdir: tmpdir

F32 = mybir.dt.float32
BF16 = mybir.dt.bfloat16

C, H, W = 256, 128, 128
CH = 2              # contraction chunks of 128
PATCH = 9
NQ = 81
YC = 16             # y rows per chunk
SLOTS = YC + 8      # 24 in2 rows resident per chunk (y0-4 .. y0+19)
XP = W + 8          # 136 padded x positions per row
NBLK, B, WXL = 4, 32, 40   # x-blocks: base 32b, 32 pixels, 40-col window
NBAND = WXL * PATCH        # 360 band cols per row
BW = 384                   # padded band row width
ROW_F = SLOTS * XP         # 3264 in2b elems per c-chunk
G = 288                    # guard cols at the front of the skew buffer
R2 = G + YC * BW           # 6432: skew buffer row width (bf16)


def ap3(t, off, dims):
    return AP(tensor=t[:].tensor, offset=t[:].offset + off, ap=dims)


def build(nc: bass.Bass, ycap=H):
    in1 = nc.declare_dram_parameter("in1", [C, H, W], F32, isOutput=False)
    in2 = nc.declare_dram_parameter("in2", [C, H, W], F32, isOutput=False)
    out = nc.declare_dram_parameter("out", [NQ, H, W], BF16, isOutput=True)

    nchunk = (ycap + YC - 1) // YC

    with TileContext(nc) as tc:
        with tc.tile_pool(name="const", bufs=1) as cpool, \
             tc.tile_pool(name="in1f", bufs=3) as p_in1f, \
             tc.tile_pool(name="in2f", bufs=3) as p_in2f, \
             tc.tile_pool(name="in1b", bufs=2) as p_in1b, \
             tc.tile_pool(name="in2b", bufs=2) as p_in2b, \
             tc.tile_pool(name="bandbuf", bufs=3) as p_band, \
             tc.tile_pool(name="bandrd", bufs=2) as p_c, \
             tc.tile_pool(name="t32", bufs=3) as p_t32, \
             tc.tile_pool(name="band_ps", bufs=5, space="PSUM") as p_ps, \
             tc.tile_pool(name="t_ps", bufs=2, space="PSUM") as p_tps, \
             tc.tile_pool(name="band_dram", bufs=3, space="DRAM") as p_bd:

            def issue_loads(k):
                y0 = k * YC
                in1f = p_in1f.tile([128, CH * YC * W], F32)
                nc.sync.dma_start(
                    out=in1f[:],
                    in_=ap3(in1, y0 * W,
                            [[H * W, 128], [128 * H * W, CH], [1, YC * W]]))
                new0, s0 = (0, 4) if k == 0 else (y0 + 4, 8)
                nrow = min(H, new0 + SLOTS - s0) - new0
                in2f = None
                if nrow > 0:
                    in2f = p_in2f.tile([128, CH * 20 * W], F32)
                    nc.sync.dma_start(
                        out=ap3(in2f, 0,
                                [[CH * 20 * W, 128], [20 * W, CH], [1, nrow * W]]),
                        in_=ap3(in2, new0 * W,
                                [[H * W, 128], [128 * H * W, CH], [1, nrow * W]]))
                return in1f, in2f, new0, s0, nrow

            def build_inputs(k):
                in1f, in2f, new0, s0, nrow = loads.pop(k)
                in1b = p_in1b.tile([128, CH * YC * W], BF16)
                nc.scalar.copy(out=in1b[:], in_=in1f[:])
                # ---- in2 row-major padded buffer: rows y0-4 .. y0+19 ----
                in2b = p_in2b.tile([128, CH * ROW_F], BF16)
                for ch in range(CH):
                    nc.vector.memset(
                        ap3(in2b, ch * ROW_F, [[CH * ROW_F, 128], [1, 4 * SLOTS]]), 0)
                    nc.vector.memset(
                        ap3(in2b, ch * ROW_F + (4 + W) * SLOTS,
                            [[CH * ROW_F, 128], [1, 4 * SLOTS]]), 0)
                if k == 0:
                    for ch in range(CH):
                        nc.vector.memset(
                            ap3(in2b, ch * ROW_F, [[CH * ROW_F, 128], [SLOTS, XP], [1, 4]]), 0)
                else:
                    nc.vector.tensor_copy(
                        ap3(in2b, 0, [[CH * ROW_F, 128], [ROW_F, CH], [SLOTS, XP], [1, 8]]),
                        ap3(prev_in2b[0], 16,
                            [[CH * ROW_F, 128], [ROW_F, CH], [SLOTS, XP], [1, 8]]))
                if nrow > 0:
                    for ch in range(CH):
                        nc.vector.tensor_copy(
                            ap3(in2b, ch * ROW_F + 4 * SLOTS + s0,
                                [[CH * ROW_F, 128], [SLOTS, W], [1, nrow]]),
                            ap3(in2f, ch * 20 * W,
                                [[CH * 20 * W, 128], [1, W], [W, nrow]]))
                stail = s0 + nrow
                if stail < SLOTS:
                    for ch in range(CH):
                        nc.vector.memset(
                            ap3(in2b, ch * ROW_F + stail,
                                [[CH * ROW_F, 128], [SLOTS, XP], [1, SLOTS - stail]]), 0)
                prev_in2b[0] = in2b
                return in1b, in2b

            prepared = {}
            ident = cpool.tile([128, 128], BF16)
            masks.make_identity(nc, ident[:])
            loads = {0: issue_loads(0)}
            if nchunk > 1:
                loads[1] = issue_loads(1)
            prev_in2b = [None]
            deferred = None
            for k in range(nchunk):
                y0 = k * YC
                if k + 2 < nchunk:
                    loads[k + 2] = issue_loads(k + 2)
                if k in prepared:
                    in1b, in2b = prepared.pop(k)
                else:
                    in1b, in2b = build_inputs(k)

                # ---- per row: matmuls + band copy ----
                bandbuf = p_band.tile([128, YC * BW], BF16)
                bandrd = p_c.tile([128, YC * 128], BF16)
                t32 = p_t32.tile([128, YC * 128], BF16)
                thalf = 0
                band_dram = p_bd.tile([128, YC * BW], BF16)
                if k < 3:
                    # first use of each bandbuf: init the per-slot junk cols
                    # (360..383) that the full-row band write reads
                    nc.vector.memset(
                        ap3(bandbuf, NBAND,
                            [[YC * BW, 128], [BW, YC], [1, BW - NBAND]]), 0)
                if k < 3:
                    # first use of each readback buffer: init the junk cols
                    # (81..127 per slot) the transposes read but the 81-col
                    # readback never writes
                    nc.vector.memset(
                        ap3(bandrd, 81, [[YC * 128, 128], [128, YC], [1, 128 - 81]]),
                        0)

                for t in range(YC):
                    if y0 + t >= ycap:
                        break
                    # interleave previous chunk's transposes into this row loop
                    if deferred is not None and t in (6, 9, 12, 15):
                        d_bandrd, d_t32, d_y0, d_ylim, d_th, d_k = deferred
                        tg = {6: 0, 9: 4, 12: 8, 15: 12}[t]
                        if tg < d_ylim:
                            tps = p_tps.tile([128, 512], BF16)
                            for dt in range(4):
                                if tg + dt >= d_ylim:
                                    break
                                nc.tensor.transpose(
                                    tps[:, dt * 128:(dt + 1) * 128],
                                    d_bandrd[:, (tg + dt) * 128:(tg + dt + 1) * 128],
                                    ident[:])
                            nc.vector.tensor_copy(
                                ap3(d_t32, d_th + tg * 128, [[YC * 128, 81], [1, 512]]),
                                tps[0:81, :])
                    ps = p_ps.tile([128, 512], F32)
                    for b in range(NBLK):
                        xb = 32 * b
                        for ch in range(CH):
                            lhsT = ap3(in1b, ch * YC * W + t * W + xb,
                                       [[CH * YC * W, 128], [1, B]])
                            # rhs col (xl, dy) = in2b row t+dy, xp xb+xl
                            rhs = ap3(in2b, ch * ROW_F + xb * SLOTS + t,
                                      [[CH * ROW_F, 128], [SLOTS, WXL], [1, PATCH]])
                            pso = AP(tensor=ps[:].tensor,
                                     offset=ps[:].offset + xb * 512,
                                     ap=[[512, B], [1, NBAND]])
                            nc.tensor.matmul(pso, lhsT, rhs,
                                             start=(ch == 0), stop=(ch == CH - 1),
                                             tile_position=(0, xb))
                    if t % 16 < 7:
                        nc.vector.tensor_copy(
                            ap3(bandbuf, t * BW, [[YC * BW, 128], [1, NBAND]]),
                            ps[:, 0:NBAND])
                    else:
                        nc.scalar.copy(
                            out=ap3(bandbuf, t * BW, [[YC * BW, 128], [1, NBAND]]),
                            in_=ps[:, 0:NBAND])
                    if t == 7:
                        # first-half band write overlaps rows 8-15
                        write_band(nc, band_dram, bandbuf, 0, 8)

                # ---- second-half band write + skewed 81-col read back ----
                write_band(nc, band_dram, bandbuf, 8, 8)
                for b in range(NBLK):
                    nc.gpsimd.dma_start(
                        out=ap3(bandrd, b * B * YC * 128,
                                [[YC * 128, B], [128, YC], [1, 81]]),
                        in_=ap3(band_dram, b * B * YC * BW,
                                [[YC * BW + 9, B], [BW, YC], [1, 81]]))

                # ---- flush previous chunk's outputs ----
                if deferred is not None:
                    flush_out(nc, out, deferred)
                deferred = (bandrd, t32, y0, min(YC, ycap - y0), thalf, k)
                if k + 1 < nchunk:
                    prepared[k + 1] = build_inputs(k + 1)

            # final chunk: transposes + outputs
            if deferred is not None:
                d_bandrd, d_t32, d_y0, d_ylim, d_th, d_k = deferred
                for tg in range(0, d_ylim, 4):
                    tps = p_tps.tile([128, 512], BF16)
                    for dt in range(4):
                        if tg + dt >= d_ylim:
                            break
                        nc.tensor.transpose(
                            tps[:, dt * 128:(dt + 1) * 128],
                            d_bandrd[:, (tg + dt) * 128:(tg + dt + 1) * 128],
                            ident[:])
                    nc.vector.tensor_copy(
                        ap3(d_t32, d_th + tg * 128, [[YC * 128, 81], [1, 512]]),
                        tps[0:81, :])
                flush_out(nc, out, deferred)
    return nc


def write_band(nc, band_dram, bandbuf, yh0, nyr):
    nc.scalar.dma_start(
        out=ap3(band_dram, yh0 * BW, [[YC * BW, 128], [1, nyr * BW]]),
        in_=ap3(bandbuf, yh0 * BW, [[YC * BW, 128], [1, nyr * BW]]))


def flush_out(nc, out, deferred):
    d_bandrd, d_t32, d_y0, d_ylim, d_th, d_k = deferred
    for u in range(PATCH):
        nc.gpsimd.dma_start(
            out=ap3(out, u * H * W + d_y0 * W,
                    [[PATCH * H * W, PATCH], [W, d_ylim], [1, W]]),
            in_=ap3(d_t32, (PATCH * u) * YC * 128,
                    [[YC * 128, PATCH], [128, d_ylim], [1, 128]]))


_cached_nc = [None]


def _get_nc():
    if _cached_nc[0] is None:
        nc = bass.Bass()
        build(nc)
        _cached_nc[0] = nc
    return _cached_nc[0]


def kernel(input1: np.ndarray, input2: np.ndarray, _trace=False) -> np.ndarray:
    from concourse.bass_utils import run_bass_kernel_spmd
    nc = _get_nc()
    n = input1.shape[0]
    in_maps = [{"in1": np.ascontiguousarray(input1[i], dtype=np.float32),
                "in2": np.ascontiguousarray(input2[i], dtype=np.float32)}
               for i in range(n)]
    r = run_bass_kernel_spmd(nc, in_maps, core_ids=list(range(n)), trace=_trace)
    outs = np.stack([r.results[i]["out"].reshape(9, 9, 128, 128) for i in range(n)])
    if _trace:
        return outs.astype(np.float32), r
    return outs.astype(np.float32)



# revision 54
# speedup vs baseline: 1.0372x; 1.0266x over previous
"""Correlation volume (9x9 displacements) kernel for 8 Trainium2 NeuronCores.

input1, input2: [8, 256, 128, 128] f32  ->  out [8, 9, 9, 128, 128] f32
Data-parallel over batch N: core n computes batch element n.

Self-contained: builds and compiles the Bass kernel on first call.
"""
import sys
if '/opt/trn_rl_repo' not in sys.path:
    sys.path.insert(0, '/opt/trn_rl_repo')

import numpy as np
import concourse.bass as bass
import concourse.mybir as mybir
import concourse.masks as masks
from concourse.ap import AP
from concourse.tile import TileContext

# ---- workaround: this walrus build allows only 1 semaphore wait per
# instruction; split Tile's multi-wait instructions into nop-carried
# single waits, and the tail drain likewise ----
import concourse.tile as _tile
import concourse.bass2jax as _b2j
import concourse.bass_utils as _bu
from concourse.vector_clock import ScopedClock as _ScopedClock

def _patched_drain_and_barrier(self, tick_clock, wait_clock):
    nc = self.nc
    probe = nc.sync.nop(nofuse=True)
    wait_clock.add_sem_waits(probe.ins, _ScopedClock({None: tick_clock.global_clock}))
    waits = list(probe.ins.sync_info.on_wait or [])
    probe.ins.sync_info.on_wait = waits[:1]
    rest = waits[1:]
    while rest:
        nop = nc.sync.nop(nofuse=True)
        if nop.ins.sync_info is None:
            nop.ins.sync_info = mybir.SyncInfo(on_wait=[], on_update=[])
        nop.ins.sync_info.on_wait = rest[:1]
        rest = rest[1:]
    nc.sync.drain()
    nc.all_engine_barrier()
    assert self.sems is not None
    popped = nc._tile_sem_poison_stack.pop()
    assert popped is self._sem_poison
    nc.clear_and_free_semaphores(list(self.sems.allocated().values()))
    nc.all_engine_barrier()

_tile.TileContext._drain_and_barrier = _patched_drain_and_barrier

_nop_counter = [0]

def _split_multi_waits(nc):
    for f in nc.m.functions:
        for blk in f.blocks:
            insts, outl, changed = list(blk.instructions), [], False
            for inst in insts:
                si = inst.sync_info
                if si is not None and si.on_wait and len(si.on_wait) > 1:
                    waits = list(si.on_wait)
                    for w in waits[:-1]:
                        _nop_counter[0] += 1
                        outl.append(mybir.InstNoOp(
                            name=f"waitsplit-{_nop_counter[0]}",
                            engine=inst.engine,
                            sync_info=mybir.SyncInfo(on_wait=[w], on_update=[])))
                    si.on_wait = waits[-1:]
                    inst.sync_info = si
                    changed = True
                outl.append(inst)
            if changed:
                blk.instructions = outl

_orig_run_via_pjrt = _b2j.run_bass_via_pjrt

def _patched_run_via_pjrt(nc, in_maps, **kw):
    _split_multi_waits(nc)
    return _orig_run_via_pjrt(nc, in_maps, **kw)

_b2j.run_bass_via_pjrt = _patched_run_via_pjrt
_bu.bass2jax = _b2j
_bu.upload_artifacts = lambda tmpdir: tmpdir

F32 = mybir.dt.float32
BF16 = mybir.dt.bfloat16

C, H, W = 256, 128, 128
CH = 2              # contraction chunks of 128
PATCH = 9
NQ = 81
YC = 16             # y rows per chunk
SLOTS = YC + 8      # 24 in2 rows resident per chunk (y0-4 .. y0+19)
XP = W + 8          # 136 padded x positions per row
NBLK, B, WXL = 4, 32, 40   # x-blocks: base 32b, 32 pixels, 40-col window
NBAND = WXL * PATCH        # 360 band cols per row
BW = 384                   # padded band row width
ROW_F = SLOTS * XP         # 3264 in2b elems per c-chunk
G = 288                    # guard cols at the front of the skew buffer
R2 = G + YC * BW           # 6432: skew buffer row width (bf16)


def ap3(t, off, dims):
    return AP(tensor=t[:].tensor, offset=t[:].offset + off, ap=dims)


def build(nc: bass.Bass, ycap=H):
    in1 = nc.declare_dram_parameter("in1", [C, H, W], F32, isOutput=False)
    in2 = nc.declare_dram_parameter("in2", [C, H, W], F32, isOutput=False)
    out = nc.declare_dram_parameter("out", [NQ, H, W], BF16, isOutput=True)

    nchunk = (ycap + YC - 1) // YC

    with TileContext(nc) as tc:
        with tc.tile_pool(name="const", bufs=1) as cpool, \
             tc.tile_pool(name="in1f", bufs=3) as p_in1f, \
             tc.tile_pool(name="in2f", bufs=3) as p_in2f, \
             tc.tile_pool(name="in1b", bufs=2) as p_in1b, \
             tc.tile_pool(name="in2b", bufs=2) as p_in2b, \
             tc.tile_pool(name="bandbuf", bufs=3) as p_band, \
             tc.tile_pool(name="bandrd", bufs=2) as p_c, \
             tc.tile_pool(name="t32", bufs=3) as p_t32, \
             tc.tile_pool(name="band_ps", bufs=5, space="PSUM") as p_ps, \
             tc.tile_pool(name="t_ps", bufs=2, space="PSUM") as p_tps, \
             tc.tile_pool(name="band_dram", bufs=3, space="DRAM") as p_bd:

            def issue_loads(k):
                y0 = k * YC
                in1f = p_in1f.tile([128, CH * YC * W], F32)
                nc.sync.dma_start(
                    out=in1f[:],
                    in_=ap3(in1, y0 * W,
                            [[H * W, 128], [128 * H * W, CH], [1, YC * W]]))
                new0, s0 = (0, 4) if k == 0 else (y0 + 4, 8)
                nrow = min(H, new0 + SLOTS - s0) - new0
                in2f = None
                if nrow > 0:
                    in2f = p_in2f.tile([128, CH * 20 * W], F32)
                    nc.sync.dma_start(
                        out=ap3(in2f, 0,
                                [[CH * 20 * W, 128], [20 * W, CH], [1, nrow * W]]),
                        in_=ap3(in2, new0 * W,
                                [[H * W, 128], [128 * H * W, CH], [1, nrow * W]]))
                return in1f, in2f, new0, s0, nrow

            ident = cpool.tile([128, 128], BF16)
            masks.make_identity(nc, ident[:])
            loads = {0: issue_loads(0)}
            if nchunk > 1:
                loads[1] = issue_loads(1)
            prev_in2b = None
            deferred = None
            for k in range(nchunk):
                y0 = k * YC
                if k + 2 < nchunk:
                    loads[k + 2] = issue_loads(k + 2)
                in1f, in2f, new0, s0, nrow = loads.pop(k)
                in1b = p_in1b.tile([128, CH * YC * W], BF16)
                nc.scalar.copy(out=in1b[:], in_=in1f[:])

                # ---- in2 row-major padded buffer: rows y0-4 .. y0+19 ----
                in2b = p_in2b.tile([128, CH * ROW_F], BF16)
                # x-pad zeroing (xp 0..3 and 132..135, all rows)
                for ch in range(CH):
                    nc.vector.memset(
                        ap3(in2b, ch * ROW_F, [[CH * ROW_F, 128], [1, 4 * SLOTS]]), 0)
                    nc.vector.memset(
                        ap3(in2b, ch * ROW_F + (4 + W) * SLOTS,
                            [[CH * ROW_F, 128], [1, 4 * SLOTS]]), 0)
                if k == 0:
                    # rows -4..-1 (slots 0..3) zero
                    for ch in range(CH):
                        nc.vector.memset(
                            ap3(in2b, ch * ROW_F, [[CH * ROW_F, 128], [SLOTS, XP], [1, 4]]), 0)
                else:
                    # slots 0..7 <- prev slots 16..23
                    nc.vector.tensor_copy(
                        ap3(in2b, 0, [[CH * ROW_F, 128], [ROW_F, CH], [SLOTS, XP], [1, 8]]),
                        ap3(prev_in2b, 16,
                            [[CH * ROW_F, 128], [ROW_F, CH], [SLOTS, XP], [1, 8]]))
                if nrow > 0:
                    for ch in range(CH):
                        nc.vector.tensor_copy(
                            ap3(in2b, ch * ROW_F + 4 * SLOTS + s0,
                                [[CH * ROW_F, 128], [SLOTS, W], [1, nrow]]),
                            ap3(in2f, ch * 20 * W,
                                [[CH * 20 * W, 128], [1, W], [W, nrow]]))
                stail = s0 + nrow
                if stail < SLOTS:
                    for ch in range(CH):
                        nc.vector.memset(
                            ap3(in2b, ch * ROW_F + stail,
                                [[CH * ROW_F, 128], [SLOTS, XP], [1, SLOTS - stail]]), 0)

                # ---- per row: matmuls + band copy ----
                bandbuf = p_band.tile([128, YC * BW], BF16)
                bandrd = p_c.tile([128, YC * 128], BF16)
                t32 = p_t32.tile([128, YC * 128], BF16)
                thalf = 0
                band_dram = p_bd.tile([128, YC * BW], BF16)
                if k < 3:
                    # first use of each bandbuf: init the per-slot junk cols
                    # (360..383) that the full-row band write reads
                    nc.vector.memset(
                        ap3(bandbuf, NBAND,
                            [[YC * BW, 128], [BW, YC], [1, BW - NBAND]]), 0)
                if k < 3:
                    # first use of each readback buffer: init the junk cols
                    # (81..127 per slot) the transposes read but the 81-col
                    # readback never writes
                    nc.vector.memset(
                        ap3(bandrd, 81, [[YC * 128, 128], [128, YC], [1, 128 - 81]]),
                        0)

                for t in range(YC):
                    if y0 + t >= ycap:
                        break
                    # interleave previous chunk's transposes into this row loop
                    if deferred is not None and t % 4 == 2:
                        d_bandrd, d_t32, d_y0, d_ylim, d_th, d_k = deferred
                        tg = (t // 4) * 4
                        if tg < d_ylim:
                            tps = p_tps.tile([128, 512], BF16)
                            for dt in range(4):
                                if tg + dt >= d_ylim:
                                    break
                                nc.tensor.transpose(
                                    tps[:, dt * 128:(dt + 1) * 128],
                                    d_bandrd[:, (tg + dt) * 128:(tg + dt + 1) * 128],
                                    ident[:])
                            nc.vector.tensor_copy(
                                ap3(d_t32, d_th + tg * 128, [[YC * 128, 81], [1, 512]]),
                                tps[0:81, :])
                    ps = p_ps.tile([128, 512], F32)
                    for b in range(NBLK):
                        xb = 32 * b
                        for ch in range(CH):
                            lhsT = ap3(in1b, ch * YC * W + t * W + xb,
                                       [[CH * YC * W, 128], [1, B]])
                            # rhs col (xl, dy) = in2b row t+dy, xp xb+xl
                            rhs = ap3(in2b, ch * ROW_F + xb * SLOTS + t,
                                      [[CH * ROW_F, 128], [SLOTS, WXL], [1, PATCH]])
                            pso = AP(tensor=ps[:].tensor,
                                     offset=ps[:].offset + xb * 512,
                                     ap=[[512, B], [1, NBAND]])
                            nc.tensor.matmul(pso, lhsT, rhs,
                                             start=(ch == 0), stop=(ch == CH - 1),
                                             tile_position=(0, xb))
                    if t % 16 < 7:
                        nc.vector.tensor_copy(
                            ap3(bandbuf, t * BW, [[YC * BW, 128], [1, NBAND]]),
                            ps[:, 0:NBAND])
                    else:
                        nc.scalar.copy(
                            out=ap3(bandbuf, t * BW, [[YC * BW, 128], [1, NBAND]]),
                            in_=ps[:, 0:NBAND])
                    if t == 7:
                        # first-half band write overlaps rows 8-15
                        write_band(nc, band_dram, bandbuf, 0, 8)

                # ---- second-half band write + skewed 81-col read back ----
                write_band(nc, band_dram, bandbuf, 8, 8)
                for b in range(NBLK):
                    nc.gpsimd.dma_start(
                        out=ap3(bandrd, b * B * YC * 128,
                                [[YC * 128, B], [128, YC], [1, 81]]),
                        in_=ap3(band_dram, b * B * YC * BW,
                                [[YC * BW + 9, B], [BW, YC], [1, 81]]))

                # ---- flush previous chunk's outputs ----
                if deferred is not None:
                    flush_out(nc, out, deferred)
                deferred = (bandrd, t32, y0, min(YC, ycap - y0), thalf, k)
                prev_in2b = in2b

            # final chunk: transposes + outputs
            if deferred is not None:
                d_bandrd, d_t32, d_y0, d_ylim, d_th, d_k = deferred
                for tg in range(0, d_ylim, 4):
                    tps = p_tps.tile([128, 512], BF16)
                    for dt in range(4):
                        if tg + dt >= d_ylim:
                            break
                        nc.tensor.transpose(
                            tps[:, dt * 128:(dt + 1) * 128],
                            d_bandrd[:, (tg + dt) * 128:(tg + dt + 1) * 128],
                            ident[:])
                    nc.vector.tensor_copy(
                        ap3(d_t32, d_th + tg * 128, [[YC * 128, 81], [1, 512]]),
                        tps[0:81, :])
                flush_out(nc, out, deferred)
    return nc


def write_band(nc, band_dram, bandbuf, yh0, nyr):
    nc.scalar.dma_start(
        out=ap3(band_dram, yh0 * BW, [[YC * BW, 128], [1, nyr * BW]]),
        in_=ap3(bandbuf, yh0 * BW, [[YC * BW, 128], [1, nyr * BW]]))


def flush_out(nc, out, deferred):
    d_bandrd, d_t32, d_y0, d_ylim, d_th, d_k = deferred
    for u in range(PATCH):
        nc.gpsimd.dma_start(
            out=ap3(out, u * H * W + d_y0 * W,
                    [[PATCH * H * W, PATCH], [W, d_ylim], [1, W]]),
            in_=ap3(d_t32, (PATCH * u) * YC * 128,
                    [[YC * 128, PATCH], [128, d_ylim], [1, 128]]))


_cached_nc = [None]


def _get_nc():
    if _cached_nc[0] is None:
        nc = bass.Bass()
        build(nc)
        _cached_nc[0] = nc
    return _cached_nc[0]


def kernel(input1: np.ndarray, input2: np.ndarray, _trace=False) -> np.ndarray:
    from concourse.bass_utils import run_bass_kernel_spmd
    nc = _get_nc()
    n = input1.shape[0]
    in_maps = [{"in1": np.ascontiguousarray(input1[i], dtype=np.float32),
                "in2": np.ascontiguousarray(input2[i], dtype=np.float32)}
               for i in range(n)]
    r = run_bass_kernel_spmd(nc, in_maps, core_ids=list(range(n)), trace=_trace)
    outs = np.stack([r.results[i]["out"].reshape(9, 9, 128, 128) for i in range(n)])
    if _trace:
        return outs.astype(np.float32), r
    return outs.astype(np.float32)



# revision 55
# speedup vs baseline: 1.0754x; 1.0368x over previous
"""Correlation volume (9x9 displacements) kernel for 8 Trainium2 NeuronCores.

input1, input2: [8, 256, 128, 128] f32  ->  out [8, 9, 9, 128, 128] f32
Data-parallel over batch N: core n computes batch element n.

Self-contained: builds and compiles the Bass kernel on first call.
"""
import sys
if '/opt/trn_rl_repo' not in sys.path:
    sys.path.insert(0, '/opt/trn_rl_repo')

import numpy as np
import concourse.bass as bass
import concourse.mybir as mybir
import concourse.masks as masks
from concourse.ap import AP
from concourse.tile import TileContext

# ---- workaround: this walrus build allows only 1 semaphore wait per
# instruction; split Tile's multi-wait instructions into nop-carried
# single waits, and the tail drain likewise ----
import concourse.tile as _tile
import concourse.bass2jax as _b2j
import concourse.bass_utils as _bu
from concourse.vector_clock import ScopedClock as _ScopedClock

def _patched_drain_and_barrier(self, tick_clock, wait_clock):
    nc = self.nc
    probe = nc.sync.nop(nofuse=True)
    wait_clock.add_sem_waits(probe.ins, _ScopedClock({None: tick_clock.global_clock}))
    waits = list(probe.ins.sync_info.on_wait or [])
    probe.ins.sync_info.on_wait = waits[:1]
    rest = waits[1:]
    while rest:
        nop = nc.sync.nop(nofuse=True)
        if nop.ins.sync_info is None:
            nop.ins.sync_info = mybir.SyncInfo(on_wait=[], on_update=[])
        nop.ins.sync_info.on_wait = rest[:1]
        rest = rest[1:]
    nc.sync.drain()
    nc.all_engine_barrier()
    assert self.sems is not None
    popped = nc._tile_sem_poison_stack.pop()
    assert popped is self._sem_poison
    nc.clear_and_free_semaphores(list(self.sems.allocated().values()))
    nc.all_engine_barrier()

_tile.TileContext._drain_and_barrier = _patched_drain_and_barrier

_nop_counter = [0]

def _split_multi_waits(nc):
    for f in nc.m.functions:
        for blk in f.blocks:
            insts, outl, changed = list(blk.instructions), [], False
            for inst in insts:
                si = inst.sync_info
                if si is not None and si.on_wait and len(si.on_wait) > 1:
                    waits = list(si.on_wait)
                    for w in waits[:-1]:
                        _nop_counter[0] += 1
                        outl.append(mybir.InstNoOp(
                            name=f"waitsplit-{_nop_counter[0]}",
                            engine=inst.engine,
                            sync_info=mybir.SyncInfo(on_wait=[w], on_update=[])))
                    si.on_wait = waits[-1:]
                    inst.sync_info = si
                    changed = True
                outl.append(inst)
            if changed:
                blk.instructions = outl

_orig_run_via_pjrt = _b2j.run_bass_via_pjrt

def _patched_run_via_pjrt(nc, in_maps, **kw):
    _split_multi_waits(nc)
    return _orig_run_via_pjrt(nc, in_maps, **kw)

_b2j.run_bass_via_pjrt = _patched_run_via_pjrt
_bu.bass2jax = _b2j
_bu.upload_artifacts = lambda tmpdir: tmpdir

F32 = mybir.dt.float32
BF16 = mybir.dt.bfloat16

C, H, W = 256, 128, 128
CH = 2              # contraction chunks of 128
PATCH = 9
NQ = 81
YC = 16             # y rows per chunk
SLOTS = YC + 8      # 24 in2 rows resident per chunk (y0-4 .. y0+19)
XP = W + 8          # 136 padded x positions per row
NBLK, B, WXL = 4, 32, 40   # x-blocks: base 32b, 32 pixels, 40-col window
NBAND = WXL * PATCH        # 360 band cols per row
BW = 384                   # padded band row width
ROW_F = SLOTS * XP         # 3264 in2b elems per c-chunk
G = 288                    # guard cols at the front of the skew buffer
R2 = G + YC * BW           # 6432: skew buffer row width (bf16)


def ap3(t, off, dims):
    return AP(tensor=t[:].tensor, offset=t[:].offset + off, ap=dims)


def build(nc: bass.Bass, ycap=H):
    in1 = nc.declare_dram_parameter("in1", [C, H, W], F32, isOutput=False)
    in2 = nc.declare_dram_parameter("in2", [C, H, W], F32, isOutput=False)
    out = nc.declare_dram_parameter("out", [NQ, H, W], BF16, isOutput=True)

    nchunk = (ycap + YC - 1) // YC

    with TileContext(nc) as tc:
        with tc.tile_pool(name="const", bufs=1) as cpool, \
             tc.tile_pool(name="in1f", bufs=3) as p_in1f, \
             tc.tile_pool(name="in2f", bufs=3) as p_in2f, \
             tc.tile_pool(name="in1b", bufs=2) as p_in1b, \
             tc.tile_pool(name="in2b", bufs=2) as p_in2b, \
             tc.tile_pool(name="bandbuf", bufs=3) as p_band, \
             tc.tile_pool(name="bandrd", bufs=2) as p_c, \
             tc.tile_pool(name="t32", bufs=3) as p_t32, \
             tc.tile_pool(name="band_ps", bufs=5, space="PSUM") as p_ps, \
             tc.tile_pool(name="t_ps", bufs=2, space="PSUM") as p_tps, \
             tc.tile_pool(name="band_dram", bufs=3, space="DRAM") as p_bd:

            def issue_loads(k):
                y0 = k * YC
                in1f = p_in1f.tile([128, CH * YC * W], F32)
                nc.sync.dma_start(
                    out=in1f[:],
                    in_=ap3(in1, y0 * W,
                            [[H * W, 128], [128 * H * W, CH], [1, YC * W]]))
                new0, s0 = (0, 4) if k == 0 else (y0 + 4, 8)
                nrow = min(H, new0 + SLOTS - s0) - new0
                in2f = None
                if nrow > 0:
                    in2f = p_in2f.tile([128, CH * 20 * W], F32)
                    nc.sync.dma_start(
                        out=ap3(in2f, 0,
                                [[CH * 20 * W, 128], [20 * W, CH], [1, nrow * W]]),
                        in_=ap3(in2, new0 * W,
                                [[H * W, 128], [128 * H * W, CH], [1, nrow * W]]))
                return in1f, in2f, new0, s0, nrow

            ident = cpool.tile([128, 128], BF16)
            masks.make_identity(nc, ident[:])
            loads = {0: issue_loads(0)}
            if nchunk > 1:
                loads[1] = issue_loads(1)
            prev_in2b = None
            deferred = None
            for k in range(nchunk):
                y0 = k * YC
                if k + 2 < nchunk:
                    loads[k + 2] = issue_loads(k + 2)
                in1f, in2f, new0, s0, nrow = loads.pop(k)
                in1b = p_in1b.tile([128, CH * YC * W], BF16)
                nc.scalar.copy(out=in1b[:], in_=in1f[:])

                # ---- in2 row-major padded buffer: rows y0-4 .. y0+19 ----
                in2b = p_in2b.tile([128, CH * ROW_F], BF16)
                # x-pad zeroing (xp 0..3 and 132..135, all rows)
                for ch in range(CH):
                    nc.vector.memset(
                        ap3(in2b, ch * ROW_F, [[CH * ROW_F, 128], [1, 4 * SLOTS]]), 0)
                    nc.vector.memset(
                        ap3(in2b, ch * ROW_F + (4 + W) * SLOTS,
                            [[CH * ROW_F, 128], [1, 4 * SLOTS]]), 0)
                if k == 0:
                    # rows -4..-1 (slots 0..3) zero
                    for ch in range(CH):
                        nc.vector.memset(
                            ap3(in2b, ch * ROW_F, [[CH * ROW_F, 128], [SLOTS, XP], [1, 4]]), 0)
                else:
                    # slots 0..7 <- prev slots 16..23
                    nc.vector.tensor_copy(
                        ap3(in2b, 0, [[CH * ROW_F, 128], [ROW_F, CH], [SLOTS, XP], [1, 8]]),
                        ap3(prev_in2b, 16,
                            [[CH * ROW_F, 128], [ROW_F, CH], [SLOTS, XP], [1, 8]]))
                if nrow > 0:
                    for ch in range(CH):
                        nc.vector.tensor_copy(
                            ap3(in2b, ch * ROW_F + 4 * SLOTS + s0,
                                [[CH * ROW_F, 128], [SLOTS, W], [1, nrow]]),
                            ap3(in2f, ch * 20 * W,
                                [[CH * 20 * W, 128], [1, W], [W, nrow]]))
                stail = s0 + nrow
                if stail < SLOTS:
                    for ch in range(CH):
                        nc.vector.memset(
                            ap3(in2b, ch * ROW_F + stail,
                                [[CH * ROW_F, 128], [SLOTS, XP], [1, SLOTS - stail]]), 0)

                # ---- per row: matmuls + band copy ----
                bandbuf = p_band.tile([128, YC * BW], BF16)
                bandrd = p_c.tile([128, YC * 128], BF16)
                t32 = p_t32.tile([128, YC * 128], BF16)
                thalf = 0
                band_dram = p_bd.tile([128, YC * BW], BF16)
                if k < 3:
                    # first use of each bandbuf: init the per-slot junk cols
                    # (360..383) that the full-row band write reads
                    nc.vector.memset(
                        ap3(bandbuf, NBAND,
                            [[YC * BW, 128], [BW, YC], [1, BW - NBAND]]), 0)
                if k < 3:
                    # first use of each readback buffer: init the junk cols
                    # (81..127 per slot) the transposes read but the 81-col
                    # readback never writes
                    nc.vector.memset(
                        ap3(bandrd, 81, [[YC * 128, 128], [128, YC], [1, 128 - 81]]),
                        0)

                for t in range(YC):
                    if y0 + t >= ycap:
                        break
                    # interleave previous chunk's transposes into this row loop
                    if deferred is not None and t % 4 == 2:
                        d_bandrd, d_t32, d_y0, d_ylim, d_th, d_k = deferred
                        tg = (t // 4) * 4
                        if tg < d_ylim:
                            tps = p_tps.tile([128, 512], BF16)
                            for dt in range(4):
                                if tg + dt >= d_ylim:
                                    break
                                nc.tensor.transpose(
                                    tps[:, dt * 128:(dt + 1) * 128],
                                    d_bandrd[:, (tg + dt) * 128:(tg + dt + 1) * 128],
                                    ident[:])
                            nc.vector.tensor_copy(
                                ap3(d_t32, d_th + tg * 128, [[YC * 128, 81], [1, 512]]),
                                tps[0:81, :])
                    ps = p_ps.tile([128, 512], F32)
                    for b in range(NBLK):
                        xb = 32 * b
                        for ch in range(CH):
                            lhsT = ap3(in1b, ch * YC * W + t * W + xb,
                                       [[CH * YC * W, 128], [1, B]])
                            # rhs col (xl, dy) = in2b row t+dy, xp xb+xl
                            rhs = ap3(in2b, ch * ROW_F + xb * SLOTS + t,
                                      [[CH * ROW_F, 128], [SLOTS, WXL], [1, PATCH]])
                            pso = AP(tensor=ps[:].tensor,
                                     offset=ps[:].offset + xb * 512,
                                     ap=[[512, B], [1, NBAND]])
                            nc.tensor.matmul(pso, lhsT, rhs,
                                             start=(ch == 0), stop=(ch == CH - 1),
                                             tile_position=(0, xb))
                    if t % 16 < 7:
                        nc.vector.tensor_copy(
                            ap3(bandbuf, t * BW, [[YC * BW, 128], [1, NBAND]]),
                            ps[:, 0:NBAND])
                    else:
                        nc.scalar.copy(
                            out=ap3(bandbuf, t * BW, [[YC * BW, 128], [1, NBAND]]),
                            in_=ps[:, 0:NBAND])

                # ---- flat band write + skewed 81-col read back ----
                nc.scalar.dma_start(
                    out=ap3(band_dram, 0, [[YC * BW, 128], [1, YC * BW]]),
                    in_=bandbuf[:, :])
                for b in range(NBLK):
                    nc.gpsimd.dma_start(
                        out=ap3(bandrd, b * B * YC * 128,
                                [[YC * 128, B], [128, YC], [1, 81]]),
                        in_=ap3(band_dram, b * B * YC * BW,
                                [[YC * BW + 9, B], [BW, YC], [1, 81]]))

                # ---- flush previous chunk's outputs ----
                if deferred is not None:
                    flush_out(nc, out, deferred)
                deferred = (bandrd, t32, y0, min(YC, ycap - y0), thalf, k)
                prev_in2b = in2b

            # final chunk: transposes + outputs
            if deferred is not None:
                d_bandrd, d_t32, d_y0, d_ylim, d_th, d_k = deferred
                for tg in range(0, d_ylim, 4):
                    tps = p_tps.tile([128, 512], BF16)
                    for dt in range(4):
                        if tg + dt >= d_ylim:
                            break
                        nc.tensor.transpose(
                            tps[:, dt * 128:(dt + 1) * 128],
                            d_bandrd[:, (tg + dt) * 128:(tg + dt + 1) * 128],
                            ident[:])
                    nc.vector.tensor_copy(
                        ap3(d_t32, d_th + tg * 128, [[YC * 128, 81], [1, 512]]),
                        tps[0:81, :])
                flush_out(nc, out, deferred)
    return nc


def flush_out(nc, out, deferred):
    d_bandrd, d_t32, d_y0, d_ylim, d_th, d_k = deferred
    for u in range(PATCH):
        nc.gpsimd.dma_start(
            out=ap3(out, u * H * W + d_y0 * W,
                    [[PATCH * H * W, PATCH], [W, d_ylim], [1, W]]),
            in_=ap3(d_t32, (PATCH * u) * YC * 128,
                    [[YC * 128, PATCH], [128, d_ylim], [1, 128]]))


_cached_nc = [None]


def _get_nc():
    if _cached_nc[0] is None:
        nc = bass.Bass()
        build(nc)
        _cached_nc[0] = nc
    return _cached_nc[0]


def kernel(input1: np.ndarray, input2: np.ndarray, _trace=False) -> np.ndarray:
    from concourse.bass_utils import run_bass_kernel_spmd
    nc = _get_nc()
    n = input1.shape[0]
    in_maps = [{"in1": np.ascontiguousarray(input1[i], dtype=np.float32),
                "in2": np.ascontiguousarray(input2[i], dtype=np.float32)}
               for i in range(n)]
    r = run_bass_kernel_spmd(nc, in_maps, core_ids=list(range(n)), trace=_trace)
    outs = np.stack([r.results[i]["out"].reshape(9, 9, 128, 128) for i in range(n)])
    if _trace:
        return outs.astype(np.float32), r
    return outs.astype(np.float32)

